# revision 6
# baseline (speedup 1.0000x reference)
"""CompressiveMemory forward (B=4, H=16, S=4096, D=64) on 8 TRN2 NeuronCores.

Sharding: core c -> batch b = c//2, heads h0 = (c%2)*8 .. h0+8 (data parallel
over B, tensor parallel over H; no cross-core communication).

Per core: 8 heads as 4 head-pairs; tensors packed [s, 128] = (h_even | h_odd).

Math per (b,h), with sigma(x) = elu(x)+1 = exp(min(x,0)) + relu(x):
  mo  = (sigma(q) @ mem0) / (sigma(q) @ z0)
  out = gate*mo + (1-gate)*att
  ret = (sigma(k) @ mem0) / (sigma(k) @ z0)
  new_states = mem0 + sigma(k)^T (v - ret)
  new_z      = z0 + sum_s sigma(k)

Key restructure for the state update (avoids transposing sigma(k)):
  sigma(k)^T ret = sigma(k)^T diag(1/den) sigma(k) mem0 = W @ mem0,
  where W = sigkr^T sigk (symmetric), sigkr = sigk * (1/den) row-scaled.
  So  new_states = mem0 + MV - W @ memB',  MV = sigk^T [v | 1] (ones col
  gives the z sums), and with z0 = c*ones per head, den = c * rowsum(sigk)
  folds into memB' = memB / c.

Device per (head-pair, group of 4 s-tiles of 128):
  PE transposes raw fp16 q -> qT4 psum; ACT relu(+/-), exp; DVE adds ->
  sigqT4 / sigk4 fp16.  M1 x4: mo4 = sigqT4_j.T @ blockdiag([mem0*gate|z0]).
  Pool rowsum -> den; DVE recip; sigkr = sigk * rr (0-stride bcast APs).
  Accumulating matmuls into one psum bank: MV (sigk^T [v|1]) and W
  (sigkr^T sigk), col-tiled per head.  Blend: out = mo*rrq + att*(1-gate)
  (the (1-gate) scale is folded into att on host, gate into mem0*gate).
  Per head-pair epilogue: P2 = W @ memB' on PE; host: mem0 + MV - P2.
Inputs are fp16 (halves DMA); PSUM accumulates fp32; output fp16.
"""
import numpy as np

import concourse.bass as bass
import concourse.mybir as mybir
import concourse.tile as tile
from concourse import bacc
from concourse.bass_utils import run_bass_kernel_spmd
from concourse.masks import make_identity

B, H, S, D = 4, 16, 4096, 64
NCORES = 8
HPC = 8          # heads per core
NHP = 4          # head-pairs per core
GRP = 4          # s-tiles of 128 per group
NT = S // 128    # 32 s-tiles per head-pair
NG = NT // GRP   # 8 groups
D2 = 2 * D       # 128
NW = D2 + 2      # 130
NV = D2 + 2      # v payload incl ones cols: [h0 v(64) | 1 | h1 v(64) | 1]
MW = 256         # per-j stride in mo4 psum tile

F32 = mybir.dt.float32
F16 = mybir.dt.float16
AF = mybir.ActivationFunctionType
OP = mybir.AluOpType
AX = mybir.AxisListType

_CACHE = {}


def _v(t, *dims):
    return bass.AP(tensor=t.tensor, offset=t.offset,
                   ap=[t.ap[0]] + [list(d) for d in dims])


def _vo(t, off, *dims):
    return bass.AP(tensor=t.tensor, offset=t.offset + off,
                   ap=[t.ap[0]] + [list(d) for d in dims])


def _build():
    nc = bacc.Bacc()
    qx = nc.declare_dram_parameter("qx", [NHP, S, D2], F16, isOutput=False)
    kx = nc.declare_dram_parameter("kx", [NHP, S, D2], F16, isOutput=False)
    vx = nc.declare_dram_parameter("vx", [NHP, S, NV], F16, isOutput=False)
    ax = nc.declare_dram_parameter("ax", [NHP, S, D2], F16, isOutput=False)
    memA = nc.declare_dram_parameter("memA", [NHP, D2, NW], F16, isOutput=False)
    memB = nc.declare_dram_parameter("memB", [NHP, D2, NW], F16, isOutput=False)
    ox = nc.declare_dram_parameter("ox", [NHP, S, D2], F16, isOutput=True)
    stMV = nc.declare_dram_parameter("stMV", [NHP, D2, D + 1], F32, isOutput=True)
    stP2 = nc.declare_dram_parameter("stP2", [NHP, D, NW], F32, isOutput=True)

    with tile.TileContext(nc) as tc:
        with (
            tc.tile_pool(name="consts", bufs=1) as consts,
            tc.tile_pool(name="io", bufs=3) as io,
            tc.tile_pool(name="oio", bufs=2) as oio,
            tc.tile_pool(name="wk", bufs=2) as wk,
            tc.tile_pool(name="psq", bufs=2, space="PSUM") as psq,
            tc.tile_pool(name="psm1", bufs=2, space="PSUM") as psm1,
            tc.tile_pool(name="psacc", bufs=1, space="PSUM") as psacc,
            tc.tile_pool(name="psp2", bufs=1, space="PSUM") as psp2,
        ):
            id16 = consts.tile([128, 128], F16, name="id16", tag="id16")
            make_identity(nc, id16)
            memA_sb = consts.tile([D2, NHP, NW], F16, name="memA_sb", tag="memA_sb")
            nc.sync.dma_start(out=memA_sb, in_=memA.ap().rearrange("h p n -> p h n"))
            memB_sb = consts.tile([D2, NHP, NW], F16, name="memB_sb", tag="memB_sb")
            nc.sync.dma_start(out=memB_sb, in_=memB.ap().rearrange("h p n -> p h n"))

            for hp in range(NHP):
                # accW bank: cols 0:65 = MV accum, cols 65:129 = W accum
                accW = psacc.tile([128, D + 1 + D], F32, name="accW", tag="accW")
                for g in range(NG):
                    s0 = g * (GRP * 128)
                    qt = io.tile([128, GRP, D2], F16, name="qt", tag="qt")
                    nc.sync.dma_start(
                        out=qt,
                        in_=qx[hp, s0:s0 + GRP * 128, :].rearrange(
                            "(j p) d -> p j d", p=128))
                    kt = io.tile([128, GRP, D2], F16, name="kt", tag="kt")
                    nc.sync.dma_start(
                        out=kt,
                        in_=kx[hp, s0:s0 + GRP * 128, :].rearrange(
                            "(j p) d -> p j d", p=128))
                    vt = io.tile([128, GRP, NV], F16, name="vt", tag="vt")
                    nc.sync.dma_start(
                        out=vt,
                        in_=vx[hp, s0:s0 + GRP * 128, :].rearrange(
                            "(j p) d -> p j d", p=128))
                    at = io.tile([128, GRP, D2], F16, name="at", tag="at")
                    nc.sync.dma_start(
                        out=at,
                        in_=ax[hp, s0:s0 + GRP * 128, :].rearrange(
                            "(j p) d -> p j d", p=128))
                    ot = oio.tile([128, GRP, D2], F16, name="ot", tag="ot")

                    # ---- q path: PE transpose fp16, sigma on [128,512]
                    qT4 = psq.tile([128, GRP, 128], F16, name="qT4", tag="qT4")
                    for j in range(GRP):
                        nc.tensor.transpose(qT4[:, j, :], qt[:, j, :], id16)
                    qT4f = qT4.rearrange("p j d -> p (j d)")
                    rq4 = wk.tile([128, GRP * 128], F16, name="rq4", tag="rq4")
                    nc.scalar.activation(out=rq4, in_=qT4f, func=AF.Relu, scale=-1.0)
                    eq4 = wk.tile([128, GRP * 128], F16, name="eq4", tag="eq4")
                    nc.scalar.activation(out=eq4, in_=rq4, func=AF.Exp, scale=-1.0)
                    rpq4 = wk.tile([128, GRP * 128], F16, name="rpq4", tag="rpq4")
                    nc.scalar.activation(out=rpq4, in_=qT4f, func=AF.Relu, scale=1.0)
                    sigqT4 = wk.tile([128, GRP, 128], F16, name="sigqT4",
                                     tag="sigqT4")
                    nc.vector.tensor_tensor(
                        out=sigqT4.rearrange("p j d -> p (j d)"), in0=rpq4,
                        in1=eq4, op=OP.add)

                    # ---- k path: sigma natural; Pool computes both relus
                    ktf = kt.rearrange("p j d -> p (j d)")
                    rk4 = wk.tile([128, GRP * 128], F16, name="rk4", tag="rk4")
                    nc.gpsimd.tensor_scalar(out=rk4, in0=ktf, scalar1=-1.0,
                                            scalar2=0.0, op0=OP.mult, op1=OP.max)
                    ek4 = wk.tile([128, GRP * 128], F16, name="ek4", tag="ek4")
                    nc.scalar.activation(out=ek4, in_=rk4, func=AF.Exp, scale=-1.0)
                    rpk4 = wk.tile([128, GRP * 128], F16, name="rpk4", tag="rpk4")
                    nc.gpsimd.tensor_scalar(out=rpk4, in0=ktf, scalar1=0.0,
                                            scalar2=None, op0=OP.max)
                    sigk4 = wk.tile([128, GRP, 128], F16, name="sigk4", tag="sigk4")
                    nc.gpsimd.tensor_tensor(
                        out=sigk4.rearrange("p j d -> p (j d)"), in0=rpk4,
                        in1=ek4, op=OP.add)

                    # ---- retrieval matmuls (q side only)
                    mo4 = psm1.tile([128, GRP, MW], F32, name="mo4", tag="mo4")
                    for j in range(GRP):
                        nc.tensor.matmul(mo4[:, j, 0:NW], sigqT4[:, j, :],
                                         memA_sb[:, hp, :], start=True, stop=True)

                    # ---- k densities: rowsum per (j, head) on Pool
                    den = wk.tile([128, GRP, 2, 1], F32, name="den", tag="den")
                    nc.vector.tensor_reduce(
                        out=den,
                        in_=sigk4.rearrange("p j (h d) -> p j h d", h=2),
                        axis=AX.X, op=OP.add)
                    rrk = wk.tile([128, GRP, 2], F32, name="rrk", tag="rrk")
                    nc.vector.reciprocal(out=rrk, in_=den.rearrange(
                        "p j h one -> p (j h one)").rearrange(
                        "p (j h) -> p j h", h=2))
                    rrk16 = wk.tile([128, GRP, 2], F16, name="rrk16", tag="rrk16")
                    nc.vector.tensor_copy(out=rrk16, in_=rrk)
                    sigkr4 = wk.tile([128, GRP, D2], F16, name="sigkr4",
                                     tag="sigkr4")
                    nc.vector.tensor_tensor(
                        out=sigkr4.rearrange("p j (h d) -> p j h d", h=2),
                        in0=sigk4.rearrange("p j (h d) -> p j h d", h=2),
                        in1=_v(rrk16, (2, GRP), (1, 2), (0, D)), op=OP.mult)

                    # ---- accumulate MV = sigk^T [v|1] and W = sigkr^T sigk
                    for j in range(GRP):
                        first = (g == 0 and j == 0)
                        last = (g == NG - 1 and j == GRP - 1)
                        nc.tensor.matmul(accW[0:D, 0:D + 1], sigk4[:, j, 0:D],
                                         vt[:, j, 0:D + 1], start=first,
                                         stop=last, tile_position=(0, 0))
                        nc.tensor.matmul(accW[D:D2, 0:D + 1], sigk4[:, j, D:D2],
                                         vt[:, j, D + 1:NV], start=first,
                                         stop=last, tile_position=(0, 64))
                        nc.tensor.matmul(accW[0:D, D + 1:], sigkr4[:, j, 0:D],
                                         sigk4[:, j, 0:D], start=first,
                                         stop=last, tile_position=(0, 0))
                        nc.tensor.matmul(accW[D:D2, D + 1:], sigkr4[:, j, D:D2],
                                         sigk4[:, j, D:D2], start=first,
                                         stop=last, tile_position=(0, 64))

                    # ---- gated output blend
                    rrq = wk.tile([128, GRP, 2], F32, name="rrq", tag="rrq")
                    nc.vector.reciprocal(
                        out=rrq, in_=_vo(mo4, D, (MW, GRP), (D + 1, 2)))
                    tmpq = wk.tile([128, GRP, 2, D], F16, name="tmpq", tag="tmpq")
                    nc.vector.tensor_tensor(
                        out=tmpq,
                        in0=_v(mo4, (MW, GRP), (D + 1, 2), (1, D)),
                        in1=_v(rrq, (2, GRP), (1, 2), (0, D)), op=OP.mult)
                    nc.vector.tensor_tensor(
                        out=ot.rearrange("p j (h d) -> p j h d", h=2),
                        in0=tmpq,
                        in1=at.rearrange("p j (h d) -> p j h d", h=2),
                        op=OP.add)

                    nc.sync.dma_start(
                        out=ox[hp, s0:s0 + GRP * 128, :].rearrange(
                            "(j p) d -> p j d", p=128),
                        in_=ot)

                # ---- head-pair epilogue: P2 = W @ memB', outputs
                mvf = wk.tile([128, D + 1], F32, name="mvf", tag="mvf")
                nc.scalar.copy(out=mvf, in_=accW[:, 0:D + 1])
                nc.sync.dma_start(out=stMV[hp, :, :], in_=mvf)
                wsb = wk.tile([128, D], F16, name="wsb", tag="wsb")
                nc.scalar.copy(out=wsb, in_=accW[:, D + 1:])
                p2 = psp2.tile([D, NW], F32, name="p2", tag="p2")
                nc.tensor.matmul(p2, wsb, memB_sb[:, hp, :], start=True, stop=True)
                p2f = wk.tile([D, NW], F32, name="p2f", tag="p2f")
                nc.scalar.copy(out=p2f, in_=p2)
                nc.sync.dma_start(out=stP2[hp, :, :], in_=p2f)

    nc.finalize()
    return nc


def _get_nc():
    if "nc" not in _CACHE:
        _CACHE["nc"] = _build()
    return _CACHE["nc"]


def _make_in_maps(q, k, v, att, betas, init_mem, init_z):
    gate = 1.0 / (1.0 + np.exp(-betas[0, :, 0, :].astype(np.float64)))
    gate = gate.astype(np.float32)                      # [H, D]
    mem0 = init_mem[0]                                  # [H, D, D]
    z0 = init_z[0, :, :, 0]                             # [H, D]
    mem0g = mem0 * gate[:, None, :]
    att_g = att * (1.0 - gate)[None, :, None, :]        # fold (1-gate) into att
    # z0 must be constant per head for the folded den path (setup uses ones)
    zc = z0[:, 0]
    assert np.allclose(z0, zc[:, None], rtol=1e-6, atol=1e-7), \
        "non-constant init_z per head not supported by this kernel"

    in_maps = []
    for c in range(NCORES):
        b = c // 2
        h0 = (c % 2) * HPC

        def pack(x, src_b=True):
            xc = x[b, h0:h0 + HPC] if src_b else x[h0:h0 + HPC]
            xc = xc.reshape(NHP, 2, S, D).transpose(0, 2, 1, 3)
            return np.ascontiguousarray(
                xc.reshape(NHP, S, D2).astype(np.float16))

        vv = np.ones((NHP, S, NV), np.float16)
        vp = v[b, h0:h0 + HPC].reshape(NHP, 2, S, D).transpose(0, 2, 1, 3)
        vv[:, :, 0:D] = vp[:, :, 0].astype(np.float16)
        vv[:, :, D + 1:NV - 1] = vp[:, :, 1].astype(np.float16)

        mA = np.zeros((NHP, D2, NW), np.float16)
        mB = np.zeros((NHP, D2, NW), np.float16)
        for hp in range(NHP):
            he, ho = h0 + 2 * hp, h0 + 2 * hp + 1
            mA[hp, 0:D, 0:D] = mem0g[he]
            mA[hp, 0:D, D] = z0[he]
            mA[hp, D:D2, D + 1:NW - 1] = mem0g[ho]
            mA[hp, D:D2, NW - 1] = z0[ho]
            mB[hp, 0:D, 0:D] = mem0[he] / zc[he]
            mB[hp, D:D2, D + 1:NW - 1] = mem0[ho] / zc[ho]

        in_maps.append({
            "qx": pack(q), "kx": pack(k), "vx": vv,
            "ax": pack(att_g),
            "memA": mA, "memB": mB,
        })
    return in_maps


def _assemble(results, init_mem, init_z):
    out = np.empty((B, H, S, D), np.float32)
    new_states = np.empty((B, H, D, D), np.float32)
    new_z = np.empty((B, H, D, 1), np.float32)
    for c in range(NCORES):
        b = c // 2
        h0 = (c % 2) * HPC
        ocore = results[c]["ox"].astype(np.float32).reshape(
            NHP, S, 2, D).transpose(0, 2, 1, 3)
        mv = results[c]["stMV"]                          # [NHP, 128, 65]
        p2 = results[c]["stP2"]                          # [NHP, 64, 130]
        for hp in range(NHP):
            for e in range(2):
                h = h0 + 2 * hp + e
                out[b, h] = ocore[hp, e]
                mv_h = mv[hp, e * D:(e + 1) * D]         # [64, 65]
                p2_h = p2[hp, :, e * (D + 1):e * (D + 1) + D]
                new_states[b, h] = init_mem[0, h] + mv_h[:, 0:D] - p2_h
                new_z[b, h, :, 0] = init_z[0, h, :, 0] + mv_h[:, D]
    return out, new_states, new_z


def run(q, k, v, attention_output, betas, init_mem, init_z, **spmd_kwargs):
    q = np.asarray(q, np.float32)
    k = np.asarray(k, np.float32)
    v = np.asarray(v, np.float32)
    att = np.asarray(attention_output, np.float32)
    betas = np.asarray(betas, np.float32)
    init_mem = np.asarray(init_mem, np.float32)
    init_z = np.asarray(init_z, np.float32)

    nc = _get_nc()
    in_maps = _make_in_maps(q, k, v, att, betas, init_mem, init_z)
    res = run_bass_kernel_spmd(nc, in_maps, list(range(NCORES)), **spmd_kwargs)
    return _assemble(res.results, init_mem, init_z), res


def kernel(q, k, v, attention_output, betas, init_mem, init_z):
    (out, new_states, new_z), _ = run(
        q, k, v, attention_output, betas, init_mem, init_z)
    return out, new_states, new_z
